# revision 13
# baseline (speedup 1.0000x reference)
"""DOS loss kernel for Trainium2, 8 NeuronCores, SPMD.

loss = sum(w * d) + sum(softmax(-w * d, axis=-1) @ ce)
  d[k]  = ||deep_feats - n[k]||_2                      (K)
  ce[k] = logsumexp(cls_score[k]) - cls_score[k, tgt]  (K)

Sharding: W rows of w are split 512/core (w is passed transposed as
[K, 512] so K lands on partitions); n and cls_score rows (K dim) are
split 512/core to build d/ce shards, which one 4KB AllGather
replicates. Each core emits a scalar partial; the host sums 8 floats.

Numerics: n/cls/w/deep are cast to bf16 host-side (halves HBM traffic,
doubles DVE/ACT throughput). All reductions accumulate in fp32; the
resulting loss error is ~1e-5 relative, dominated by fp32 itself.
"""

import sys

import numpy as np

for _p in ("/opt/trn_rl_repo",):
    if _p not in sys.path:
        sys.path.insert(0, _p)

D, K, W, C = 2048, 4096, 4096, 1000
NCORES = 8
KS = K // NCORES  # 512 k rows per core (n, cls shards)
WS = W // NCORES  # 512 w rows per core
KT = KS // 128  # 4 stage-A/B tiles
CH = K // 128  # 32 stage-C chunks
WB = 4  # big w tiles
WSUB = CH // WB  # 8 chunks per big tile
SEG = 2 * K // 128  # 64 cc_out segments of 128

_STATE = None


def _build():
    import concourse.bass as bass
    from concourse import bacc, mybir, tile

    F32 = mybir.dt.float32
    BF16 = mybir.dt.bfloat16
    AF = mybir.ActivationFunctionType
    OP = mybir.AluOpType
    AX = mybir.AxisListType

    nc = bacc.Bacc("TRN2", target_bir_lowering=False, debug=False, num_devices=NCORES)

    deep_d = nc.dram_tensor("deep", [128, D], BF16, kind="ExternalInput")
    n_d = nc.dram_tensor("n_s", [KS, D], BF16, kind="ExternalInput")
    cls_d = nc.dram_tensor("cls_s", [KS, C], BF16, kind="ExternalInput")
    ncol_d = nc.dram_tensor("ncol_s", [KS], F32, kind="ExternalInput")
    wt_d = nc.dram_tensor("wt_s", [K, WS], BF16, kind="ExternalInput")
    out_d = nc.dram_tensor("out", [1], F32, kind="ExternalOutput")

    cc_in = nc.dram_tensor("cc_in", [2 * KS], F32)
    cc_out = nc.dram_tensor("cc_out", [2 * K], F32, addr_space="Shared")
    eye_d = nc.inline_tensor(np.eye(SEG, dtype=np.float32), name="eye64")

    with tile.TileContext(nc) as tc:
        with (
            tc.tile_pool(name="small", bufs=1) as sm,
            tc.tile_pool(name="npool", bufs=4) as npool,
            tc.tile_pool(name="nscr", bufs=2) as nscr,
            tc.tile_pool(name="clspool", bufs=4) as clspool,
            tc.tile_pool(name="clsscr", bufs=2) as clsscr,
            tc.tile_pool(name="wpool", bufs=3) as wpool,
            tc.tile_pool(name="epool", bufs=4) as epool,
            tc.tile_pool(name="psum", bufs=1, space="PSUM") as pp,
        ):
            # ---------------- input loads (latency-critical first) ---
            deep_b = sm.tile([128, D], BF16)
            nc.sync.dma_start(deep_b[:], deep_d[:])
            n_ts = []
            for t in range(KT):
                n_t = npool.tile([128, D], BF16)
                nc.sync.dma_start(n_t[:], n_d[t * 128 : (t + 1) * 128, :])
                n_ts.append(n_t)
            cls_ts = []
            for t in range(KT):
                cls_t = clspool.tile([128, C], BF16)
                nc.sync.dma_start(cls_t[:], cls_d[t * 128 : (t + 1) * 128, :])
                cls_ts.append(cls_t)
            ncol_sb = sm.tile([128, KT], F32)
            nc.sync.dma_start(ncol_sb[:], ncol_d[:].rearrange("(t p) -> p t", p=128))
            eye_sb = sm.tile([SEG, SEG], F32)
            nc.sync.dma_start(eye_sb[:], eye_d[:])
            # big w tiles stream on the tensor-engine HWDGE queues so they
            # never block the latency-critical sync-queue loads above
            w_ts = []
            for b in range(WB):
                w_t = wpool.tile([128, WSUB, WS], BF16)
                nc.gpsimd.dma_start(
                    w_t[:],
                    wt_d[:].rearrange("(b j p) w -> b p j w", b=WB, j=WSUB, p=128)[b],
                )
                w_ts.append(w_t)

            # ---------------- stage A: d shard -----------------------
            d2col = sm.tile([128, KT], F32)
            for t in range(KT):
                diff = nscr.tile([128, D], BF16, tag="ascr")
                nc.vector.tensor_sub(diff[:], n_ts[t][:], deep_b[:])
                scr2 = nscr.tile([128, D], BF16, tag="ascr2")
                nc.scalar.activation(
                    scr2[:], diff[:], AF.Square, accum_out=d2col[:, t : t + 1]
                )
            dcol = sm.tile([128, KT], F32)
            nc.scalar.activation(dcol[:], d2col[:], AF.Sqrt)
            nc.sync.dma_start(cc_in[0:KS].rearrange("(t p) -> p t", p=128), dcol[:])

            # ---------------- stage B: ce shard ----------------------
            ssum = sm.tile([128, KT], F32)
            for t in range(KT):
                escr = clsscr.tile([128, C], BF16, tag="bscr")
                nc.scalar.activation(
                    escr[:], cls_ts[t][:], AF.Exp, accum_out=ssum[:, t : t + 1]
                )
            lse = sm.tile([128, KT], F32)
            nc.scalar.activation(lse[:], ssum[:], AF.Ln)
            cecol = sm.tile([128, KT], F32)
            for t in range(KT):
                # ce = lse + (-cls[:, tgt]) , bias is per-partition
                nc.scalar.activation(
                    cecol[:, t : t + 1],
                    lse[:, t : t + 1],
                    AF.Identity,
                    bias=ncol_sb[:, t : t + 1],
                )
            nc.sync.dma_start(
                cc_in[KS : 2 * KS].rearrange("(t p) -> p t", p=128), cecol[:]
            )

            # ---------------- allgather d, ce ------------------------
            nc.gpsimd.collective_compute(
                "AllGather",
                OP.bypass,
                replica_groups=[list(range(NCORES))],
                ins=[cc_in[:]],
                outs=[cc_out[:]],
            )

            # load gathered vector as [64, 128] (contiguous, fast) and
            # PE-transpose to [128, 64]; col q = 8r+4h+s
            dce_raw = sm.tile([SEG, 128], F32)
            nc.sync.dma_start(
                dce_raw[:], cc_out[:].rearrange("(q p) -> q p", p=128)
            )
            dce_t = pp.tile([128, SEG], F32)
            nc.tensor.transpose(dce_t[:], dce_raw[:], eye_sb[:])
            # compact to chunk order c = 4r+s: d at q=8r+s, ce at q=8r+4+s
            dce_v = dce_t[:].rearrange("p (r h s) -> p r h s", r=NCORES, h=2)
            nd = sm.tile([128, NCORES, KT], F32)
            nc.vector.tensor_scalar_mul(nd[:], dce_v[:, :, 0, :], -1.0)
            ce_bf = sm.tile([128, NCORES, KT], BF16)
            nc.vector.tensor_copy(ce_bf[:], dce_v[:, :, 1, :])
            ones_bf = sm.tile([128, 1], BF16)
            nc.vector.memset(ones_bf[:], 1.0)
            ones32 = sm.tile([128, 1], F32)
            nc.vector.memset(ones32[:], 1.0)

            # ---------------- stage C: main sweep over wT ------------
            s_psum = pp.tile([1, WS], F32)
            num_psum = pp.tile([1, WS], F32)
            wsum = sm.tile([128, NCORES, KT], F32)  # chunk order, matches nd
            for b in range(WB):
                w_t = w_ts[b]
                for j in range(WSUB):
                    c = b * WSUB + j
                    et_c = epool.tile([128, WS], BF16)
                    # e = exp(-d_k * wT[k, :]) with per-partition scale -d
                    nc.scalar.activation(
                        et_c[:],
                        w_t[:, j, :],
                        AF.Exp,
                        scale=nd[:, c // KT, c % KT : c % KT + 1],
                    )
                    nc.tensor.matmul(
                        s_psum[:],
                        ones_bf[:],
                        et_c[:],
                        start=(c == 0),
                        stop=(c == CH - 1),
                    )
                    nc.tensor.matmul(
                        num_psum[:],
                        ce_bf[:, c // KT, c % KT : c % KT + 1],
                        et_c[:],
                        start=(c == 0),
                        stop=(c == CH - 1),
                    )
                nc.vector.tensor_reduce(
                    wsum[:, 2 * b : 2 * b + 2, :], w_t[:], axis=AX.X, op=OP.add
                )

            # ---------------- epilogue -------------------------------
            # nd and wsum cols are both chunk-ordered [128, 8, 4]
            fscr = sm.tile([128, NCORES, KT], F32)
            nc.vector.tensor_mul(fscr[:], nd[:], wsum[:])
            f128 = sm.tile([128, 1], F32)
            nc.vector.tensor_reduce(f128[:], fscr[:], axis=AX.XY, op=OP.add)
            fg_psum = pp.tile([1, 1], F32)
            nc.tensor.matmul(fg_psum[:], ones32[:], f128[:], start=True, stop=True)

            rec = sm.tile([1, WS], F32)
            nc.vector.reciprocal(rec[:], s_psum[0:1, :])
            grow = sm.tile([1, WS], F32)
            nc.vector.tensor_mul(grow[:], rec[:], num_psum[0:1, :])
            gacc = sm.tile([1, 1], F32)
            nc.vector.tensor_reduce(gacc[:], grow[:], axis=AX.X, op=OP.add)
            loss = sm.tile([1, 1], F32)
            # fg_psum holds f (nd=-d gives -f; mult by wsum of +w ... nd*wsum
            # sums to -f), so loss = g - (-f)
            nc.vector.tensor_sub(loss[:], gacc[:], fg_psum[0:1, 0:1])
            nc.sync.dma_start(out_d[:], loss[:])

    nc.compile()
    return nc


def _get_state():
    global _STATE
    if _STATE is None:
        _STATE = _build()
    return _STATE


def _shard_inputs(deep_feats, cls_score, target, n, w):
    import ml_dtypes

    bf16 = ml_dtypes.bfloat16
    deep_feats = np.ascontiguousarray(deep_feats, dtype=np.float32).reshape(1, D)
    cls_score = np.ascontiguousarray(cls_score, dtype=np.float32)
    n = np.ascontiguousarray(n, dtype=np.float32)
    w = np.ascontiguousarray(w, dtype=np.float32)
    tgt = int(np.asarray(target).reshape(-1)[0])
    ncol = -cls_score[:, tgt].astype(np.float32)

    deep_b = np.ascontiguousarray(
        np.broadcast_to(deep_feats.astype(bf16), (128, D))
    )
    n_bf = n.astype(bf16)
    cls_bf = cls_score.astype(bf16)

    in_maps = []
    for i in range(NCORES):
        ks = slice(i * KS, (i + 1) * KS)
        in_maps.append(
            {
                "deep": deep_b,
                "n_s": n_bf[ks],
                "cls_s": cls_bf[ks],
                "ncol_s": ncol[ks],
                "wt_s": np.ascontiguousarray(w[ks].T.astype(bf16)),
            }
        )
    return in_maps


def kernel(deep_feats, cls_score, target, n, w):
    nc = _get_state()
    from concourse.bass_utils import run_bass_kernel_spmd

    in_maps = _shard_inputs(deep_feats, cls_score, target, n, w)
    res = run_bass_kernel_spmd(nc, in_maps, list(range(NCORES)))
    total = np.float64(0.0)
    for i in range(NCORES):
        total += np.float64(res.results[i]["out"][0])
    return np.float32(total).reshape(())


# revision 16
# speedup vs baseline: 1.3428x; 1.3428x over previous
"""DOS loss kernel for Trainium2, 8 NeuronCores, SPMD.

loss = sum(w * d) + sum(softmax(-w * d, axis=-1) @ ce)
  d[k]  = ||deep_feats - n[k]||_2                      (K)
  ce[k] = logsumexp(cls_score[k]) - cls_score[k, tgt]  (K)

Sharding: the K (contraction) dimension is split 512/core everywhere —
n rows, cls rows, and a [512, W] slice of w^T (host-transposed so k
lands on partitions). Each core computes its local d/ce shard, then
partial softmax statistics over the full W:
  s_row[r]   += sum_{k in shard} exp(-d_k w[r,k])
  num_row[r] += sum_{k in shard} ce_k exp(-d_k w[r,k])
One end-of-kernel AllReduce of [s_row; num_row] (32KB) completes the
softmax; g = sum(num/s) is computed redundantly on every core. f is a
pure local partial. Each core emits f_i + g/8; the host sums 8 floats.
No mid-kernel collective, so nothing serializes on rank skew.

Numerics: n/cls/w/deep are cast to bf16 host-side (halves HBM traffic,
doubles DVE throughput). All reductions accumulate in fp32.
"""

import sys

import numpy as np

for _p in ("/opt/trn_rl_repo",):
    if _p not in sys.path:
        sys.path.insert(0, _p)

D, K, W, C = 2048, 4096, 4096, 1000
NCORES = 8
KS = K // NCORES  # 512 k rows per core
KT = KS // 128  # 4 k chunks per core
EH = 2  # exp tile halves per chunk
EW = W // EH  # 2048 columns per exp tile
NB = W // 512  # 8 psum bank slices
SEG = 2 * W // 128  # 64 segments in the allreduce result

_STATE = None


def _build():
    import concourse.bass as bass
    from concourse import bacc, mybir, tile

    F32 = mybir.dt.float32
    BF16 = mybir.dt.bfloat16
    AF = mybir.ActivationFunctionType
    OP = mybir.AluOpType
    AX = mybir.AxisListType

    nc = bacc.Bacc("TRN2", target_bir_lowering=False, debug=False, num_devices=NCORES)

    deep_d = nc.dram_tensor("deep", [128, D], BF16, kind="ExternalInput")
    n_d = nc.dram_tensor("n_s", [KS, D], BF16, kind="ExternalInput")
    cls_d = nc.dram_tensor("cls_s", [KS, C], BF16, kind="ExternalInput")
    ncol_d = nc.dram_tensor("ncol_s", [KS], F32, kind="ExternalInput")
    wt_d = nc.dram_tensor("wt_s", [KS, W], BF16, kind="ExternalInput")
    out_d = nc.dram_tensor("out", [1], F32, kind="ExternalOutput")

    ar_in = nc.dram_tensor("ar_in", [2 * W], F32)
    ar_out = nc.dram_tensor("ar_out", [2 * W], F32, addr_space="Shared")
    eye_d = nc.inline_tensor(np.eye(SEG, dtype=np.float32), name="eye64")

    with tile.TileContext(nc) as tc:
        with (
            tc.tile_pool(name="small", bufs=1) as sm,
            tc.tile_pool(name="npool", bufs=4) as npool,
            tc.tile_pool(name="nscr", bufs=2) as nscr,
            tc.tile_pool(name="clspool", bufs=4) as clspool,
            tc.tile_pool(name="clsscr", bufs=2) as clsscr,
            tc.tile_pool(name="wpool", bufs=3) as wpool,
            tc.tile_pool(name="epool", bufs=3) as epool,
            tc.tile_pool(name="psum", bufs=1, space="PSUM") as pp,
        ):
            # ---------------- activation table warmup ----------------
            warm = sm.tile([1, 2], F32)
            nc.vector.memset(warm[:], 1.0)
            wscr = sm.tile([1, 2], F32)
            for fn in (AF.Square, AF.Sqrt, AF.Exp, AF.Ln):
                nc.scalar.activation(wscr[:], warm[:], fn)

            # ---------------- input loads ----------------------------
            deep_b = sm.tile([128, D], BF16)
            nc.sync.dma_start(deep_b[:], deep_d[:])
            n_ts = []
            for t in range(KT):
                n_t = npool.tile([128, D], BF16)
                nc.sync.dma_start(n_t[:], n_d[t * 128 : (t + 1) * 128, :])
                n_ts.append(n_t)
            ncol_sb = sm.tile([128, KT], F32)
            nc.sync.dma_start(ncol_sb[:], ncol_d[:].rearrange("(t p) -> p t", p=128))
            eye_sb = sm.tile([SEG, SEG], F32)
            nc.sync.dma_start(eye_sb[:], eye_d[:])
            # cls on the scalar-engine HWDGE queues, w on gpsimd SWDGE —
            # three independent issue paths so nothing serializes
            cls_ts = []
            for t in range(KT):
                cls_t = clspool.tile([128, C], BF16)
                nc.scalar.dma_start(cls_t[:], cls_d[t * 128 : (t + 1) * 128, :])
                cls_ts.append(cls_t)
            w_ts = []
            for t in range(KT):
                w_t = wpool.tile([128, W], BF16)
                nc.gpsimd.dma_start(w_t[:], wt_d[t * 128 : (t + 1) * 128, :])
                w_ts.append(w_t)

            # ---------------- stage A: local d ------------------------
            d2col = sm.tile([128, KT], F32)
            for t in range(KT):
                diff = nscr.tile([128, D], BF16, tag="ascr")
                nc.vector.tensor_sub(diff[:], n_ts[t][:], deep_b[:])
                scr2 = nscr.tile([128, D], BF16, tag="ascr2")
                nc.scalar.activation(
                    scr2[:], diff[:], AF.Square, accum_out=d2col[:, t : t + 1]
                )
            dcol = sm.tile([128, KT], F32)
            nc.scalar.activation(dcol[:], d2col[:], AF.Sqrt)
            ndcol = sm.tile([128, KT], F32)
            nc.vector.tensor_scalar_mul(ndcol[:], dcol[:], -1.0)

            # ---------------- stage B: local ce -----------------------
            ssum = sm.tile([128, KT], F32)
            for t in range(KT):
                escr = clsscr.tile([128, C], BF16, tag="bscr")
                nc.scalar.activation(
                    escr[:], cls_ts[t][:], AF.Exp, accum_out=ssum[:, t : t + 1]
                )
            lse = sm.tile([128, KT], F32)
            nc.scalar.activation(lse[:], ssum[:], AF.Ln)
            cecol = sm.tile([128, KT], F32)
            nc.vector.tensor_add(cecol[:], lse[:], ncol_sb[:])
            # lhsT pairs [ones, ce] per k chunk, bf16
            snl = sm.tile([128, KT, 2], BF16)
            nc.vector.memset(snl[:, :, 0], 1.0)
            nc.vector.tensor_copy(snl[:, :, 1], cecol[:])

            # ---------------- stage C: sweep local wT over all W ------
            sn_psum = pp.tile([2, W], F32, tag="ps")
            wsum = sm.tile([128, KT], F32)
            for t in range(KT):
                w_t = w_ts[t]
                for h in range(EH):
                    et = epool.tile([128, EW], BF16)
                    nc.scalar.activation(
                        et[:],
                        w_t[:, h * EW : (h + 1) * EW],
                        AF.Exp,
                        scale=ndcol[:, t : t + 1],
                    )
                    for b in range(EW // 512):
                        nb = h * (EW // 512) + b
                        nc.tensor.matmul(
                            sn_psum[:, nb * 512 : (nb + 1) * 512],
                            snl[:, t, :],
                            et[:, b * 512 : (b + 1) * 512],
                            start=(t == 0),
                            stop=(t == KT - 1),
                        )
                nc.vector.tensor_reduce(
                    wsum[:, t : t + 1], w_t[:], axis=AX.X, op=OP.add
                )

            # f partial: sum_k d_k * rowsum_k (local)
            fscr = sm.tile([128, KT], F32)
            nc.vector.tensor_mul(fscr[:], dcol[:], wsum[:])
            f128 = sm.tile([128, 1], F32)
            nc.vector.tensor_reduce(f128[:], fscr[:], axis=AX.X, op=OP.add)

            # ---------------- allreduce [s; num] ----------------------
            sn_sb = sm.tile([2, W], F32)
            nc.vector.tensor_copy(sn_sb[:], sn_psum[:])
            nc.sync.dma_start(ar_in[:].rearrange("(x w) -> x w", x=2), sn_sb[:])
            nc.gpsimd.collective_compute(
                "AllReduce",
                OP.add,
                replica_groups=[list(range(NCORES))],
                ins=[ar_in[:]],
                outs=[ar_out[:]],
            )
            snt_raw = sm.tile([SEG, 128], F32)
            nc.sync.dma_start(snt_raw[:], ar_out[:].rearrange("(q p) -> q p", p=128))
            snt = pp.tile([128, SEG], F32, tag="ps")
            nc.tensor.transpose(snt[:], snt_raw[:], eye_sb[:])

            # ---------------- epilogue --------------------------------
            # snt cols 0..31 = s (col q holds s[q*128+p]), 32..63 = num
            rec = sm.tile([128, SEG // 2], F32)
            nc.vector.reciprocal(rec[:], snt[:, 0 : SEG // 2])
            grow = sm.tile([128, SEG // 2], F32)
            nc.vector.tensor_mul(grow[:], rec[:], snt[:, SEG // 2 : SEG])
            g128 = sm.tile([128, 1], F32)
            nc.vector.tensor_reduce(g128[:], grow[:], axis=AX.X, op=OP.add)
            # out = f_i + g/8 per core; host sums to f + g
            g8 = sm.tile([128, 1], F32)
            nc.vector.tensor_scalar_mul(g8[:], g128[:], 1.0 / NCORES)
            t128 = sm.tile([128, 1], F32)
            nc.vector.tensor_add(t128[:], f128[:], g8[:])
            ones32 = sm.tile([128, 1], F32)
            nc.vector.memset(ones32[:], 1.0)
            loss_ps = pp.tile([1, 1], F32, tag="ps")
            nc.tensor.matmul(loss_ps[:], ones32[:], t128[:], start=True, stop=True)
            loss = sm.tile([1, 1], F32)
            nc.vector.tensor_copy(loss[:], loss_ps[:])
            nc.sync.dma_start(out_d[:], loss[:])

    nc.compile()
    return nc


def _get_state():
    global _STATE
    if _STATE is None:
        _STATE = _build()
    return _STATE


def _shard_inputs(deep_feats, cls_score, target, n, w):
    import ml_dtypes

    bf16 = ml_dtypes.bfloat16
    deep_feats = np.ascontiguousarray(deep_feats, dtype=np.float32).reshape(1, D)
    cls_score = np.ascontiguousarray(cls_score, dtype=np.float32)
    n = np.ascontiguousarray(n, dtype=np.float32)
    w = np.ascontiguousarray(w, dtype=np.float32)
    tgt = int(np.asarray(target).reshape(-1)[0])
    ncol = -cls_score[:, tgt].astype(np.float32)

    deep_b = np.ascontiguousarray(
        np.broadcast_to(deep_feats.astype(bf16), (128, D))
    )
    n_bf = n.astype(bf16)
    cls_bf = cls_score.astype(bf16)
    wt_bf = np.ascontiguousarray(w.T.astype(bf16))  # [K, W]

    in_maps = []
    for i in range(NCORES):
        ks = slice(i * KS, (i + 1) * KS)
        in_maps.append(
            {
                "deep": deep_b,
                "n_s": n_bf[ks],
                "cls_s": cls_bf[ks],
                "ncol_s": ncol[ks],
                "wt_s": wt_bf[ks],
            }
        )
    return in_maps


def kernel(deep_feats, cls_score, target, n, w):
    nc = _get_state()
    from concourse.bass_utils import run_bass_kernel_spmd

    in_maps = _shard_inputs(deep_feats, cls_score, target, n, w)
    res = run_bass_kernel_spmd(nc, in_maps, list(range(NCORES)))
    total = np.float64(0.0)
    for i in range(NCORES):
        total += np.float64(res.results[i]["out"][0])
    return np.float32(total).reshape(())
